# revision 7
# baseline (speedup 1.0000x reference)
"""Balanced BCE loss on 8 Trainium2 NeuronCores.

loss = -sum_i [ beta_i * sum_j(t_ij * ln(p_ij))
                + (1-beta_i) * sum_j((1-t_ij) * ln(1-p_ij)) ]
beta_i = 1 - mean_j(t_ij)

Per-core row statistics (8 batch rows per core):
  S=sum(t)  A=sum(t*lnp)  C=sum(t*ln1mp)  B=sum(ln1mp)
host combines: loss = -sum_rows[ beta*A + (1-beta)*(B-C) ], beta = 1-S/N

Engine assignment per row tile [128, F]:
  - ACT: lnp = Ln(p) bf16; ln1mp = Ln(1-p) bf16
  - DVE: cast t->bf16 (2x); m1 = t*lnp, m2 = t*ln1mp (bf16 TT, 2x)
  - PE: one-hot-weight chunk matmuls accumulate all four stats into
    per-virtual-row PSUM partitions: psX[12, CH] += onehot_v^T @ chunk.
    A single accumulation group per stat spans the whole kernel.
  - End: four DVE X-reduces [12, CH] -> [12, 1] produce the output.
All input DMAs are issued upfront; every row tile is resident in SBUF
(no recycling stalls).  The last batch row is processed in 4 column
quarters so the tail after the final DMA is short.
"""

from contextlib import ExitStack

import numpy as np

import concourse.bass as bass
import concourse.mybir as mybir
import concourse.tile as tile
from concourse import bacc
from concourse.bass_utils import run_bass_kernel_spmd

B, N = 64, 262144
NCORES = 8
ROWS = B // NCORES  # rows per core
P = 128  # SBUF partitions
F = N // P  # 2048 elements per partition per row
QF = F // 4  # quarter-row width for the tail row
NV = 11  # virtual rows: 7 full + 4 quarters
CH = 256  # matmul moving-dim chunk

AF = mybir.ActivationFunctionType
ALU = mybir.AluOpType
AX = mybir.AxisListType
f32 = mybir.dt.float32
bf16 = mybir.dt.bfloat16

# test.py can flip this to capture an NTFF profile of the run
TRACE = False
LAST = None  # BassKernelResults of the most recent kernel() call


def _emit(tc, out_ap, inp_ap, tgt_ap):
    nc = tc.nc
    rows = ROWS

    with ExitStack() as ctx:
        io_pool = ctx.enter_context(tc.tile_pool(name="io", bufs=rows - 1))
        ioq_pool = ctx.enter_context(tc.tile_pool(name="ioq", bufs=4))
        bf_pool = ctx.enter_context(tc.tile_pool(name="bf", bufs=2))
        tb_pool = ctx.enter_context(tc.tile_pool(name="tb", bufs=3))
        psum_pool = ctx.enter_context(tc.tile_pool(name="ps", bufs=1, space="PSUM"))
        singles = ctx.enter_context(tc.tile_pool(name="const", bufs=1))

        # one-hot stationary matrices: block v is [128, NV+1] bf16 with
        # column v all-ones (routes vrow v's partition sums to psum row v)
        oh = singles.tile([P, NV * (NV + 1)], bf16, tag="oh")
        nc.vector.memset(oh[:], 0.0)
        for v in range(NV):
            nc.vector.memset(oh[:, v * (NV + 1) + v : v * (NV + 1) + v + 1], 1.0)
        out_sb = singles.tile([NV + 1, 4], f32, tag="out_sb")

        inp3 = inp_ap.rearrange("r (p f) -> p r f", p=P)
        tgt3 = tgt_ap.rearrange("r (p f) -> p r f", p=P)

        # stat psum tiles [NV+1, CH] f32, one accumulation group each
        psS = psum_pool.tile([NV + 1, CH], f32, tag="psS", name="psS")
        psB = psum_pool.tile([NV + 1, CH], f32, tag="psB", name="psB")
        psA = psum_pool.tile([NV + 1, CH], f32, tag="psA", name="psA")
        psC = psum_pool.tile([NV + 1, CH], f32, tag="psC", name="psC")

        # virtual rows: (col, dram_row, col_offset, width)
        vrows = [(r, r, 0, F) for r in range(rows - 1)]
        vrows += [(7 + q, rows - 1, q * QF, QF) for q in range(4)]

        # all input DMAs upfront on the SP HWDGE queue; every tile resident
        ptiles, ttiles = [], []
        for col, r, off, w in vrows:
            pool = io_pool if w == F else ioq_pool
            pp = pool.tile([P, w], f32, tag=f"p{w}", name=f"pp_{col}")
            nc.sync.dma_start(pp[:], inp3[:, r, off : off + w])
            ptiles.append(pp)
            tt = pool.tile([P, w], f32, tag=f"t{w}", name=f"tt_{col}")
            nc.sync.dma_start(tt[:], tgt3[:, r, off : off + w])
            ttiles.append(tt)

        first_v = vrows[0][0]
        last_v = vrows[-1][0]
        for i, (col, r, off, w) in enumerate(vrows):
            p_t = ptiles[i][:]
            t_t = ttiles[i][:]
            nch = w // CH

            logp = bf_pool.tile([P, w], bf16, tag=f"logp{w}")
            nc.scalar.activation(logp[:], p_t, AF.Ln)
            l1mp = bf_pool.tile([P, w], bf16, tag=f"l1mp{w}")
            nc.scalar.activation(l1mp[:], p_t, AF.Ln, scale=-1.0, bias=1.0)

            tb = tb_pool.tile([P, w], bf16, tag=f"tb{w}")
            nc.vector.tensor_copy(tb[:], t_t)
            m1 = bf_pool.tile([P, w], bf16, tag=f"m{w}")
            nc.vector.tensor_mul(m1[:], tb[:], logp[:])
            m2 = bf_pool.tile([P, w], bf16, tag=f"m{w}")
            nc.vector.tensor_mul(m2[:], tb[:], l1mp[:])

            ohv = oh[:, col * (NV + 1) : (col + 1) * (NV + 1)]
            for ps, src in ((psS, tb), (psB, l1mp), (psA, m1), (psC, m2)):
                for c in range(nch):
                    nc.tensor.matmul(
                        ps[:, :],
                        ohv,
                        src[:, c * CH : (c + 1) * CH],
                        start=(col == first_v and c == 0),
                        stop=(col == last_v and c == nch - 1),
                        skip_group_check=True,
                    )

        # second level: X-reduce each stat [NV+1, CH] -> [NV+1, 1]
        for j, ps in enumerate((psS, psB, psA, psC)):
            nc.vector.tensor_reduce(
                out_sb[:, j : j + 1], ps[:, :], axis=AX.X, op=ALU.add
            )
        nc.sync.dma_start(out_ap, out_sb[:])


_PROG_CACHE = {}


def _build_program():
    key = (ROWS, N)
    if key not in _PROG_CACHE:
        nc = bacc.Bacc("TRN2", target_bir_lowering=False, debug=False)
        inp = nc.dram_tensor("input", [ROWS, N], f32, kind="ExternalInput").ap()
        tgt = nc.dram_tensor("target", [ROWS, N], f32, kind="ExternalInput").ap()
        out = nc.dram_tensor("partials", [NV + 1, 4], f32, kind="ExternalOutput").ap()
        with tile.TileContext(nc) as tc:
            _emit(tc, out, inp, tgt)
        nc.finalize()
        _PROG_CACHE[key] = nc
    return _PROG_CACHE[key]


def kernel(input, target):
    global LAST
    input = np.ascontiguousarray(np.asarray(input))
    target = np.ascontiguousarray(np.asarray(target))
    assert input.shape == (B, N) and target.shape == (B, N)

    nc = _build_program()
    in_maps = [
        {
            "input": input[c * ROWS : (c + 1) * ROWS],
            "target": target[c * ROWS : (c + 1) * ROWS],
        }
        for c in range(NCORES)
    ]
    res = run_bass_kernel_spmd(nc, in_maps, core_ids=list(range(NCORES)), trace=TRACE)
    LAST = res

    total = np.float64(0.0)
    for c in range(NCORES):
        part = res.results[c]["partials"].astype(np.float64)  # [NV+1, 4]
        # vrows 0..6 are batch rows 0..6; vrows 7..10 are row-7 quarters
        S = np.concatenate([part[:7, 0], [part[7:11, 0].sum()]])
        Bv = np.concatenate([part[:7, 1], [part[7:11, 1].sum()]])
        A = np.concatenate([part[:7, 2], [part[7:11, 2].sum()]])
        C = np.concatenate([part[:7, 3], [part[7:11, 3].sum()]])
        beta = 1.0 - S / N
        total += np.sum(beta * A + (1.0 - beta) * (Bv - C))
    return np.float32(-total)
